# revision 13
# baseline (speedup 1.0000x reference)
# Bahdanau (additive) attention kernel for Trainium2, 8 NeuronCores.
#
# reference math (per batch b):
#   enc_proj = enc[b] @ W1.T                  # [S, Dd]
#   dec_proj = dec[b] @ W2.T                  # [Dd]
#   res      = tanh(enc_proj + dec_proj)      # [S, Dd]
#   scores   = res @ v                        # [S]
#   w        = softmax(scores)                # [S]   (mask is all-False per spec)
#   context  = w @ enc[b]                     # [De]
#
# Sharding: batch dim (64) split across 8 cores (8 batches/core), weights
# replicated.  Device layout strategy:
#   - enc loaded in natural [s, e] layout (contiguous HBM reads), PE-transposed
#     to [e, s] tiles for the main matmul (contraction dim must be on
#     partitions for both matmul operands).
#   - main matmul computes enc_projT [d, s] with lhsT = W1T tiles
#     (float32r: full-rate 1 cycle/row at N=512), accumulating e-chunks in PSUM.
#   - tanh fused with the per-partition dec_proj bias on the scalar engine,
#     evacuating PSUM -> SBUF.
#   - scores via matmul with v-chunks as the stationary operand (M=1),
#     accumulating d-chunks into PSUM [1, 512] per s-chunk.
#   - softmax without max-subtraction (|scores| <= ||v||_1 ~ 18, exp is safe
#     in fp32), Z via exp(accum_out) + ones-matmul partition reduction.
#   - scores transposed (PE) to [s-partition, chunk] columns so the context
#     matmul can use exp(scores) columns as stationary operands against the
#     natural-layout enc tiles still resident in SBUF.

import numpy as np

B, S, DE, DD = 64, 2048, 512, 512
NCORES = 8
BPC = B // NCORES      # batches per core
P = 128
NSC = S // 512         # s-chunks of 512 per batch (4)
NSS = 4                # s-subchunks of 128 per s-chunk
NEC = DE // P          # e-chunks (4)
NDC = DD // P          # d-chunks (4)

# dtype used for the big matmuls: float32r streams at 1 cycle/row (N>=256)
# vs plain float32 at 4 cycles/row.  Set to "float32" to trade 4x PE time
# for exact fp32 matmul numerics.
MM_DTYPE = "float32r"

# packed-constants layout (free-dim column offsets in the [128, CF_TOT] tile)
CF_W1T = 0                      # W1.T packed [p, ec*512 + d]   (2048 cols)
CF_W2T = CF_W1T + NEC * DD      # W2.T packed [p, kc*512 + e]   (2048 cols)
CF_VC = CF_W2T + NDC * DD       # v packed    [p, dc]           (4 cols)
CF_ID = CF_VC + NDC             # identity    [p, 128]
CF_DECT = CF_ID + P             # dec.T packed [p, kc*8 + b]    (32 cols)
CF_TOT = CF_DECT + NDC * BPC    # 4260

_PROGRAM_CACHE = {}


def _build_program():
    import concourse.mybir as mybir
    import concourse.tile as tile
    from concourse import bacc

    f32 = mybir.dt.float32
    mmdt = getattr(mybir.dt, MM_DTYPE)
    AF = mybir.ActivationFunctionType

    # float32r contract (BIR verifier): any tensor consumed by an FP32r
    # matmul must be *produced* as float32r — so the DMA'd weight tiles, the
    # transpose outputs, and the activation outputs feeding matmuls are all
    # declared float32r.  float32r has the same 4-byte layout as fp32
    # (np.float32 on the host); engines round on write.

    nc = bacc.Bacc("TRN2", target_bir_lowering=False, debug=False)

    enc_d = nc.dram_tensor("enc", [BPC, S, DE], mmdt, kind="ExternalInput").ap()
    consts_d = nc.dram_tensor("consts", [P, CF_TOT], mmdt, kind="ExternalInput").ap()

    aw_d = nc.dram_tensor("aw", [BPC, S], f32, kind="ExternalOutput").ap()
    ctx_d = nc.dram_tensor("ctx", [BPC, DD], f32, kind="ExternalOutput").ap()

    from contextlib import ExitStack

    with tile.TileContext(nc) as tc, ExitStack() as ctx:
        singles = ctx.enter_context(tc.tile_pool(name="singles", bufs=1))
        encpool = ctx.enter_context(tc.tile_pool(name="encpool", bufs=6))
        encTpool = ctx.enter_context(tc.tile_pool(name="encTpool", bufs=8))
        tanhpool = ctx.enter_context(tc.tile_pool(name="tanhpool", bufs=6))
        smalls = ctx.enter_context(tc.tile_pool(name="smalls", bufs=2))
        tpsp = ctx.enter_context(tc.tile_pool(name="tpsp", bufs=2, space="PSUM"))
        mmpsp = ctx.enter_context(tc.tile_pool(name="mmpsp", bufs=2, space="PSUM"))
        scpsp = ctx.enter_context(tc.tile_pool(name="scpsp", bufs=1, space="PSUM"))
        wcpsp = ctx.enter_context(tc.tile_pool(name="wcpsp", bufs=1, space="PSUM"))
        smallps = ctx.enter_context(tc.tile_pool(name="smallps", bufs=1, space="PSUM"))
        dppsp = ctx.enter_context(tc.tile_pool(name="dppsp", bufs=1, space="PSUM"))

        # ---- load ALL constants with ONE DMA ----
        # A matmul with 4-byte weights lowers through the LDWEIGHTS struct,
        # which supports only ONE sync-wait command.  Packing every constant
        # into a single DMA (single semaphore) keeps each PE instruction at
        # <= 1 fresh wait (later waits on the same semaphore are elided).
        consts_sb = singles.tile([P, CF_TOT], mmdt)
        nc.sync.dma_start(consts_sb, consts_d)

        def w1t_tile(ec, dc):
            o = CF_W1T + ec * DD + dc * P
            return consts_sb[:, o:o + P]

        def w2t_tile(kc, ec):
            o = CF_W2T + kc * DD + ec * P
            return consts_sb[:, o:o + P]

        vcols_sb = consts_sb[:, CF_VC:CF_VC + NDC]
        ident_sb = consts_sb[:, CF_ID:CF_ID + P]

        def dect_view(kc):
            o = CF_DECT + kc * BPC
            return consts_sb[:, o:o + BPC]

        # ---- dec_proj (one-time): dprj_sb[p, ec, b] = dec_proj[b, ec*128+p] ----
        dprj_sb = singles.tile([P, NDC, BPC], f32)
        for ec in range(NDC):
            dpps = dppsp.tile([P, 512], f32, tag="dpps")
            for kc in range(NDC):
                nc.tensor.matmul(
                    dpps[:, :BPC],
                    lhsT=w2t_tile(kc, ec),
                    rhs=dect_view(kc),
                    start=(kc == 0),
                    stop=(kc == NDC - 1),
                )
            nc.vector.tensor_copy(out=dprj_sb[:, ec, :], in_=dpps[:, :BPC])

        # ---- main per-batch loop ----
        for b in range(BPC):
            enc_nats = []
            # scores rows accumulate here in SBUF, evacuated per s-chunk
            scrow_sb = smalls.tile([1, S], f32, tag="scrow")
            for sc in range(NSC):
                # load enc s-chunk in natural layout: [p, ss, e]
                enc_nat = encpool.tile([P, NSS, DE], mmdt, tag="enc")
                nc.sync.dma_start(
                    enc_nat,
                    enc_d[b, sc * 512:(sc + 1) * 512, :].rearrange(
                        "(ss p) e -> p ss e", p=P
                    ),
                )
                enc_nats.append(enc_nat)

                # transpose to encT [e, s] tiles
                encTs = []
                for ec in range(NEC):
                    tps = tpsp.tile([P, 512], mmdt, tag="tps")
                    for ss in range(NSS):
                        nc.tensor.transpose(
                            tps[:, ss * P:(ss + 1) * P],
                            enc_nat[:, ss, ec * P:(ec + 1) * P],
                            ident_sb,
                        )
                    encT = encTpool.tile([P, 512], mmdt, tag="encT")
                    nc.vector.tensor_copy(out=encT, in_=tps)
                    encTs.append(encT)

                # main matmul: enc_projT [d, s] ; tanh(+dec_proj bias)
                tanhs = []
                for dc in range(NDC):
                    mmps = mmpsp.tile([P, 512], f32, tag="mmps")
                    for ec in range(NEC):
                        nc.tensor.matmul(
                            mmps,
                            lhsT=w1t_tile(ec, dc),
                            rhs=encTs[ec],
                            start=(ec == 0),
                            stop=(ec == NEC - 1),
                        )
                    tanh_sb = tanhpool.tile([P, 512], mmdt, tag="tanh")
                    nc.scalar.activation(
                        out=tanh_sb,
                        in_=mmps,
                        func=AF.Tanh,
                        bias=dprj_sb[:, dc, b:b + 1],
                    )
                    tanhs.append(tanh_sb)

                # scores for this s-chunk: [1, 512] accumulated over d-chunks
                scps = scpsp.tile([1, 512], f32, tag="scps")
                for dc in range(NDC):
                    nc.tensor.matmul(
                        scps,
                        lhsT=vcols_sb[:, dc:dc + 1],
                        rhs=tanhs[dc],
                        start=(dc == 0),
                        stop=(dc == NDC - 1),
                    )
                # evacuate to SBUF (scalar engine, closer to PSUM)
                nc.scalar.activation(
                    out=scrow_sb[:, sc * 512:(sc + 1) * 512],
                    in_=scps,
                    func=AF.Copy,
                )

            # ---- softmax pieces ----
            # transpose scores into column layout [s % 128, chunk]
            wcps = wcpsp.tile([P, 16], f32, tag="wcps")
            for g in range(16):
                nc.tensor.transpose(
                    wcps[:, g:g + 1],
                    scrow_sb[:, g * P:(g + 1) * P],
                    ident_sb[0:1, 0:1].bitcast(f32),
                )
            # exp over the column layout (feeds the context matmul)
            wcols_sb = smalls.tile([P, 16], mmdt, tag="wcols")
            nc.scalar.activation(out=wcols_sb, in_=wcps, func=AF.Exp)

            # row-layout exp with fused free-dim sum => Z, then 1/Z
            aw_sb = smalls.tile([1, S], f32, tag="awrow")
            z_sb = smalls.tile([1, 1], f32, tag="z")
            nc.scalar.activation(
                out=aw_sb, in_=scrow_sb, func=AF.Exp, accum_out=z_sb
            )
            recip_sb = smalls.tile([1, 1], f32, tag="recip")
            nc.vector.reciprocal(out=recip_sb, in_=z_sb)

            # attention-weights output row: exp(scores) * (1/Z)
            nc.vector.tensor_scalar_mul(out=aw_sb, in0=aw_sb, scalar1=recip_sb)
            nc.sync.dma_start(aw_d[b:b + 1, :], aw_sb)

            # ---- context: sum_s w[s] * enc[s, :] ----
            ctxps = smallps.tile([1, DD], f32, tag="ctxps")
            for g in range(16):
                sc, j = g // NSS, g % NSS
                nc.tensor.matmul(
                    ctxps,
                    lhsT=wcols_sb[:, g:g + 1],
                    rhs=enc_nats[sc][:, j, :],
                    start=(g == 0),
                    stop=(g == 15),
                )
            ctx_sb = smalls.tile([1, DD], f32, tag="ctx")
            nc.vector.tensor_scalar_mul(out=ctx_sb, in0=ctxps, scalar1=recip_sb)
            nc.sync.dma_start(ctx_d[b:b + 1, :], ctx_sb)

    nc.compile()
    return nc


def _get_program():
    if "nc" not in _PROGRAM_CACHE:
        _PROGRAM_CACHE["nc"] = _build_program()
    return _PROGRAM_CACHE["nc"]


def _pack_consts(W1, W2, v, dec_slice):
    c = np.zeros((P, CF_TOT), np.float32)
    w1t = np.ascontiguousarray(W1.T)  # [e, d]
    c[:, CF_W1T:CF_W1T + NEC * DD] = (
        w1t.reshape(NEC, P, DD).transpose(1, 0, 2).reshape(P, NEC * DD))
    w2t = np.ascontiguousarray(W2.T)  # [d, e]
    c[:, CF_W2T:CF_W2T + NDC * DD] = (
        w2t.reshape(NDC, P, DD).transpose(1, 0, 2).reshape(P, NDC * DD))
    c[:, CF_VC:CF_VC + NDC] = v.reshape(NDC, P).T
    c[:, CF_ID:CF_ID + P] = np.eye(P, dtype=np.float32)
    dect = np.ascontiguousarray(dec_slice.T)  # [d, b]
    c[:, CF_DECT:CF_DECT + NDC * BPC] = (
        dect.reshape(NDC, P, BPC).transpose(1, 0, 2).reshape(P, NDC * BPC))
    return c


def _shard_inputs(decoder_hidden, encoder_outputs, W1, W2, v):
    in_maps = []
    for c in range(NCORES):
        lo, hi = c * BPC, (c + 1) * BPC
        in_maps.append({
            "enc": np.ascontiguousarray(encoder_outputs[lo:hi]),
            "consts": _pack_consts(W1, W2, v, decoder_hidden[lo:hi]),
        })
    return in_maps


def kernel(decoder_hidden, encoder_outputs, mask, W1, W2, v):
    from concourse.bass_utils import run_bass_kernel_spmd

    decoder_hidden = np.asarray(decoder_hidden, dtype=np.float32)
    encoder_outputs = np.asarray(encoder_outputs, dtype=np.float32)
    W1 = np.asarray(W1, dtype=np.float32)
    W2 = np.asarray(W2, dtype=np.float32)
    v = np.asarray(v, dtype=np.float32)
    # mask is all-False per the problem spec (fill: zeros); softmax unaffected.

    nc = _get_program()
    in_maps = _shard_inputs(decoder_hidden, encoder_outputs, W1, W2, v)
    res = run_bass_kernel_spmd(nc, in_maps, core_ids=list(range(NCORES)))

    aw = np.concatenate([r["aw"] for r in res.results], axis=0)
    ctx = np.concatenate([r["ctx"] for r in res.results], axis=0)
    return aw, ctx


if __name__ == "__main__":
    import reference

    inputs = {k: np.asarray(v) for k, v in reference.setup_inputs().items()}
    out = kernel(**inputs)
    print("aw", out[0].shape, "ctx", out[1].shape)


# revision 14
# speedup vs baseline: 18.0911x; 18.0911x over previous
# Bahdanau (additive) attention kernel for Trainium2, 8 NeuronCores.
#
# reference math (per batch b):
#   enc_proj = enc[b] @ W1.T                  # [S, Dd]
#   dec_proj = dec[b] @ W2.T                  # [Dd]
#   res      = tanh(enc_proj + dec_proj)      # [S, Dd]
#   scores   = res @ v                        # [S]
#   w        = softmax(scores)                # [S]   (mask is all-False per spec)
#   context  = w @ enc[b]                     # [De]
#
# Sharding: batch dim (64) split across 8 cores (8 batches/core), weights
# replicated.  Device layout strategy:
#   - enc loaded in natural [s, e] layout (contiguous HBM reads), PE-transposed
#     to [e, s] tiles for the main matmul (contraction dim must be on
#     partitions for both matmul operands).
#   - main matmul computes enc_projT [d, s] with lhsT = W1T tiles
#     (float32r: full-rate 1 cycle/row at N=512), accumulating e-chunks in PSUM.
#   - tanh fused with the per-partition dec_proj bias on the scalar engine,
#     evacuating PSUM -> SBUF.
#   - scores via matmul with v-chunks as the stationary operand (M=1),
#     accumulating d-chunks into PSUM [1, 512] per s-chunk.
#   - softmax without max-subtraction (|scores| <= ||v||_1 ~ 18, exp is safe
#     in fp32), Z via exp(accum_out) + ones-matmul partition reduction.
#   - scores transposed (PE) to [s-partition, chunk] columns so the context
#     matmul can use exp(scores) columns as stationary operands against the
#     natural-layout enc tiles still resident in SBUF.

import numpy as np

B, S, DE, DD = 64, 2048, 512, 512
NCORES = 8
BPC = B // NCORES      # batches per core
P = 128
NSC = S // 512         # s-chunks of 512 per batch (4)
NSS = 4                # s-subchunks of 128 per s-chunk
NEC = DE // P          # e-chunks (4)
NDC = DD // P          # d-chunks (4)

# dtype used for the big matmuls: float32r streams at 1 cycle/row (N>=256)
# vs plain float32 at 4 cycles/row.  Set to "float32" to trade 4x PE time
# for exact fp32 matmul numerics.
MM_DTYPE = "float32r"

# packed-constants layout (free-dim column offsets in the [128, CF_TOT] tile)
CF_W1T = 0                      # W1.T packed [p, ec*512 + d]   (2048 cols)
CF_W2T = CF_W1T + NEC * DD      # W2.T packed [p, kc*512 + e]   (2048 cols)
CF_VC = CF_W2T + NDC * DD       # v packed    [p, dc]           (4 cols)
CF_ID = CF_VC + NDC             # identity    [p, 128]
CF_DECT = CF_ID + P             # dec.T packed [p, kc*8 + b]    (32 cols)
CF_TOT = CF_DECT + NDC * BPC    # 4260

_PROGRAM_CACHE = {}


def _build_program(repeat=1):
    import concourse.mybir as mybir
    import concourse.tile as tile
    from concourse import bacc

    f32 = mybir.dt.float32
    mmdt = getattr(mybir.dt, MM_DTYPE)
    AF = mybir.ActivationFunctionType

    # float32r contract (BIR verifier): any tensor consumed by an FP32r
    # matmul must be *produced* as float32r — so the DMA'd weight tiles, the
    # transpose outputs, and the activation outputs feeding matmuls are all
    # declared float32r.  float32r has the same 4-byte layout as fp32
    # (np.float32 on the host); engines round on write.

    nc = bacc.Bacc("TRN2", target_bir_lowering=False, debug=False)

    enc_d = nc.dram_tensor("enc", [BPC, S, DE], mmdt, kind="ExternalInput").ap()
    consts_d = nc.dram_tensor("consts", [P, CF_TOT], mmdt, kind="ExternalInput").ap()

    aw_d = nc.dram_tensor("aw", [BPC, S], f32, kind="ExternalOutput").ap()
    ctx_d = nc.dram_tensor("ctx", [BPC, DD], f32, kind="ExternalOutput").ap()

    from contextlib import ExitStack

    with tile.TileContext(nc) as tc, ExitStack() as ctx:
        singles = ctx.enter_context(tc.tile_pool(name="singles", bufs=1))
        encpool = ctx.enter_context(tc.tile_pool(name="encpool", bufs=6))
        encTpool = ctx.enter_context(tc.tile_pool(name="encTpool", bufs=8))
        tanhpool = ctx.enter_context(tc.tile_pool(name="tanhpool", bufs=6))
        smalls = ctx.enter_context(tc.tile_pool(name="smalls", bufs=2))
        tpsp = ctx.enter_context(tc.tile_pool(name="tpsp", bufs=2, space="PSUM"))
        mmpsp = ctx.enter_context(tc.tile_pool(name="mmpsp", bufs=2, space="PSUM"))
        scpsp = ctx.enter_context(tc.tile_pool(name="scpsp", bufs=1, space="PSUM"))
        wcpsp = ctx.enter_context(tc.tile_pool(name="wcpsp", bufs=1, space="PSUM"))
        smallps = ctx.enter_context(tc.tile_pool(name="smallps", bufs=1, space="PSUM"))
        dppsp = ctx.enter_context(tc.tile_pool(name="dppsp", bufs=1, space="PSUM"))

        # ---- load ALL constants with ONE DMA ----
        # A matmul with 4-byte weights lowers through the LDWEIGHTS struct,
        # which supports only ONE sync-wait command.  Packing every constant
        # into a single DMA (single semaphore) keeps each PE instruction at
        # <= 1 fresh wait (later waits on the same semaphore are elided).
        consts_sb = singles.tile([P, CF_TOT], mmdt)
        nc.sync.dma_start(consts_sb, consts_d)

        def w1t_tile(ec, dc):
            o = CF_W1T + ec * DD + dc * P
            return consts_sb[:, o:o + P]

        def w2t_tile(kc, ec):
            o = CF_W2T + kc * DD + ec * P
            return consts_sb[:, o:o + P]

        vcols_sb = consts_sb[:, CF_VC:CF_VC + NDC]
        ident_sb = consts_sb[:, CF_ID:CF_ID + P]

        def dect_view(kc):
            o = CF_DECT + kc * BPC
            return consts_sb[:, o:o + BPC]

        # ---- dec_proj (one-time): dprj_sb[p, ec, b] = dec_proj[b, ec*128+p] ----
        dprj_sb = singles.tile([P, NDC, BPC], f32)
        for ec in range(NDC):
            dpps = dppsp.tile([P, 512], f32, tag="dpps")
            for kc in range(NDC):
                nc.tensor.matmul(
                    dpps[:, :BPC],
                    lhsT=w2t_tile(kc, ec),
                    rhs=dect_view(kc),
                    start=(kc == 0),
                    stop=(kc == NDC - 1),
                )
            nc.vector.tensor_copy(out=dprj_sb[:, ec, :], in_=dpps[:, :BPC])

        # ---- main per-batch loop ----
        # (repeat>1 re-runs the whole batch loop; used only for timing by
        #  test.py — outputs are overwritten identically each repetition)
        for b in [bb for _ in range(repeat) for bb in range(BPC)]:
            enc_nats = []
            # scores rows accumulate here in SBUF, evacuated per s-chunk
            scrow_sb = smalls.tile([1, S], f32, tag="scrow")
            for sc in range(NSC):
                # load enc s-chunk in natural layout: [p, ss, e]
                enc_nat = encpool.tile([P, NSS, DE], mmdt, tag="enc")
                nc.sync.dma_start(
                    enc_nat,
                    enc_d[b, sc * 512:(sc + 1) * 512, :].rearrange(
                        "(ss p) e -> p ss e", p=P
                    ),
                )
                enc_nats.append(enc_nat)

                # transpose to encT [e, s] tiles
                encTs = []
                for ec in range(NEC):
                    tps = tpsp.tile([P, 512], mmdt, tag="tps")
                    for ss in range(NSS):
                        nc.tensor.transpose(
                            tps[:, ss * P:(ss + 1) * P],
                            enc_nat[:, ss, ec * P:(ec + 1) * P],
                            ident_sb,
                        )
                    encT = encTpool.tile([P, 512], mmdt, tag="encT")
                    nc.vector.tensor_copy(out=encT, in_=tps)
                    encTs.append(encT)

                # main matmul: enc_projT [d, s] ; tanh(+dec_proj bias)
                tanhs = []
                for dc in range(NDC):
                    mmps = mmpsp.tile([P, 512], f32, tag="mmps")
                    for ec in range(NEC):
                        nc.tensor.matmul(
                            mmps,
                            lhsT=w1t_tile(ec, dc),
                            rhs=encTs[ec],
                            start=(ec == 0),
                            stop=(ec == NEC - 1),
                        )
                    tanh_sb = tanhpool.tile([P, 512], mmdt, tag="tanh")
                    nc.scalar.activation(
                        out=tanh_sb,
                        in_=mmps,
                        func=AF.Tanh,
                        bias=dprj_sb[:, dc, b:b + 1],
                    )
                    tanhs.append(tanh_sb)

                # scores for this s-chunk: [1, 512] accumulated over d-chunks
                scps = scpsp.tile([1, 512], f32, tag="scps")
                for dc in range(NDC):
                    nc.tensor.matmul(
                        scps,
                        lhsT=vcols_sb[:, dc:dc + 1],
                        rhs=tanhs[dc],
                        start=(dc == 0),
                        stop=(dc == NDC - 1),
                    )
                # evacuate to SBUF (scalar engine, closer to PSUM)
                nc.scalar.activation(
                    out=scrow_sb[:, sc * 512:(sc + 1) * 512],
                    in_=scps,
                    func=AF.Copy,
                )

            # ---- softmax pieces ----
            # transpose scores into column layout [s % 128, chunk]
            wcps = wcpsp.tile([P, 16], f32, tag="wcps")
            for g in range(16):
                nc.tensor.transpose(
                    wcps[:, g:g + 1],
                    scrow_sb[:, g * P:(g + 1) * P],
                    ident_sb[0:1, 0:1].bitcast(f32),
                )
            # exp over the column layout (feeds the context matmul)
            wcols_sb = smalls.tile([P, 16], mmdt, tag="wcols")
            nc.scalar.activation(out=wcols_sb, in_=wcps, func=AF.Exp)

            # row-layout exp with fused free-dim sum => Z, then 1/Z
            aw_sb = smalls.tile([1, S], f32, tag="awrow")
            z_sb = smalls.tile([1, 1], f32, tag="z")
            nc.scalar.activation(
                out=aw_sb, in_=scrow_sb, func=AF.Exp, accum_out=z_sb
            )
            recip_sb = smalls.tile([1, 1], f32, tag="recip")
            nc.vector.reciprocal(out=recip_sb, in_=z_sb)

            # attention-weights output row: exp(scores) * (1/Z)
            nc.vector.tensor_scalar_mul(out=aw_sb, in0=aw_sb, scalar1=recip_sb)
            nc.sync.dma_start(aw_d[b:b + 1, :], aw_sb)

            # ---- context: sum_s w[s] * enc[s, :] ----
            ctxps = smallps.tile([1, DD], f32, tag="ctxps")
            for g in range(16):
                sc, j = g // NSS, g % NSS
                nc.tensor.matmul(
                    ctxps,
                    lhsT=wcols_sb[:, g:g + 1],
                    rhs=enc_nats[sc][:, j, :],
                    start=(g == 0),
                    stop=(g == 15),
                )
            ctx_sb = smalls.tile([1, DD], f32, tag="ctx")
            nc.vector.tensor_scalar_mul(out=ctx_sb, in0=ctxps, scalar1=recip_sb)
            nc.sync.dma_start(ctx_d[b:b + 1, :], ctx_sb)

    nc.compile()
    return nc


def _get_program(repeat=1):
    if repeat not in _PROGRAM_CACHE:
        _PROGRAM_CACHE[repeat] = _build_program(repeat)
    return _PROGRAM_CACHE[repeat]


def _pack_consts(W1, W2, v, dec_slice):
    c = np.zeros((P, CF_TOT), np.float32)
    w1t = np.ascontiguousarray(W1.T)  # [e, d]
    c[:, CF_W1T:CF_W1T + NEC * DD] = (
        w1t.reshape(NEC, P, DD).transpose(1, 0, 2).reshape(P, NEC * DD))
    w2t = np.ascontiguousarray(W2.T)  # [d, e]
    c[:, CF_W2T:CF_W2T + NDC * DD] = (
        w2t.reshape(NDC, P, DD).transpose(1, 0, 2).reshape(P, NDC * DD))
    c[:, CF_VC:CF_VC + NDC] = v.reshape(NDC, P).T
    c[:, CF_ID:CF_ID + P] = np.eye(P, dtype=np.float32)
    dect = np.ascontiguousarray(dec_slice.T)  # [d, b]
    c[:, CF_DECT:CF_DECT + NDC * BPC] = (
        dect.reshape(NDC, P, BPC).transpose(1, 0, 2).reshape(P, NDC * BPC))
    return c


def _shard_inputs(decoder_hidden, encoder_outputs, W1, W2, v):
    in_maps = []
    for c in range(NCORES):
        lo, hi = c * BPC, (c + 1) * BPC
        in_maps.append({
            "enc": np.ascontiguousarray(encoder_outputs[lo:hi]),
            "consts": _pack_consts(W1, W2, v, decoder_hidden[lo:hi]),
        })
    return in_maps


def kernel(decoder_hidden, encoder_outputs, mask, W1, W2, v):
    from concourse.bass_utils import run_bass_kernel_spmd

    decoder_hidden = np.asarray(decoder_hidden, dtype=np.float32)
    encoder_outputs = np.asarray(encoder_outputs, dtype=np.float32)
    W1 = np.asarray(W1, dtype=np.float32)
    W2 = np.asarray(W2, dtype=np.float32)
    v = np.asarray(v, dtype=np.float32)
    # mask is all-False per the problem spec (fill: zeros); softmax unaffected.

    nc = _get_program()
    in_maps = _shard_inputs(decoder_hidden, encoder_outputs, W1, W2, v)
    res = run_bass_kernel_spmd(nc, in_maps, core_ids=list(range(NCORES)))

    aw = np.concatenate([r["aw"] for r in res.results], axis=0)
    ctx = np.concatenate([r["ctx"] for r in res.results], axis=0)
    return aw, ctx


if __name__ == "__main__":
    import reference

    inputs = {k: np.asarray(v) for k, v in reference.setup_inputs().items()}
    out = kernel(**inputs)
    print("aw", out[0].shape, "ctx", out[1].shape)
